# revision 15
# baseline (speedup 1.0000x reference)
"""DRMMTKS scoring kernel for 8 Trainium2 NeuronCores.

Strategy (vocab-sharded):
  - Each core owns a 6250-row slice of the embedding table (shipped bf16),
    normalizes it on device, and writes it to a DRAM scratch table
    (rows padded to 384 elems; a 301st coordinate carries the pad marker).
  - Doc tokens are bucketed by vocab slice on the host; each core gathers its
    tokens' normalized embeddings straight from the DRAM table with the SWDGE
    gather-transpose instruction (4 batches = 4*cap tokens per call), landing
    them in [emb, token] (matmul rhs) layout.
  - Queries are host-gathered (pure data movement), normalized + transposed on
    device; cosine similarities are computed with col-tiled matmuls
    (4 batches concurrently in the 128x128 PE array).
  - Pad tokens point at a dummy table row whose marker coordinate is -1e30
    (queries carry 1.0 there), so padded slots come out of the matmul as
    -1e30 and never reach the top-k.
  - Cosine rows are copied out of PSUM as bf16; per-core top-24 candidates
    (3 rounds of DVE max8 + match_replace) are exchanged with an AllToAll
    (each core sends exactly the 4 quads each peer's tail needs), then each
    core merges 8x24 candidates and computes the tanh-FFW + IDF-softmax tail
    for its own 16 batches.
"""

import sys

sys.path.insert(0, "/opt/trn_rl_repo")

import numpy as np
import ml_dtypes

import concourse.bass as bass
from concourse import bacc, library_config
import concourse.mybir as mybir
from concourse.tile import TileContext
from concourse.bass_utils import run_bass_kernel_spmd

# Problem shapes (hardcoded per contest contract)
V, EMB = 50000, 300
B, TQ, TD, TOPK = 128, 32, 4096, 20
NC = 8
VS = V // NC                 # 6250 vocab rows per core
NRANK = 49                   # ranks holding real vocab rows (6272 capacity)
RANKS = 50                   # +1 rank for the dummy pad row
VPAD = NRANK * 128           # 6272
DUMMY = VPAD                 # dummy row index (zero vector, mask coord -1e30)
EB = 384                     # stored row length in bf16 elems (768 bytes)
ROWB = EB * 2                # row bytes
BLOC = B // NC               # 16 batches per core in the tail
F32 = mybir.dt.float32
BF16 = mybir.dt.bfloat16
I32 = mybir.dt.int32
I16 = mybir.dt.int16
AX = mybir.AxisListType.X
MUL = mybir.AluOpType.mult
ADD = mybir.AluOpType.add
BYP = mybir.AluOpType.bypass

NEG_PAD = -1.0e30            # cos value of padded slots
NEG_REP = -2.0e30            # match_replace fill

_cache = {}

LAST_RESULTS = None          # BassKernelResults of the most recent run


def _build(cap, sim=False):
    """Trace + compile the per-core program. cap = token slots per batch."""
    nidx = 8 * cap           # gathered tokens per group (8 batches)
    assert nidx % 128 == 0 and cap % 32 == 0
    nch = cap // 2           # cos free-dim chunk (<=512)
    assert nch <= 512

    nc = bacc.Bacc("TRN2", target_bir_lowering=False, debug=False, num_devices=NC)

    tslice = nc.dram_tensor("tslice", [VPAD, EMB], BF16, kind="ExternalInput")
    qrows = nc.dram_tensor("qrows", [B * TQ, EMB], BF16, kind="ExternalInput")
    gidx = nc.dram_tensor("gidx", [128, 16 * nidx // 16], I16, kind="ExternalInput")
    idf = nc.dram_tensor("idf", [BLOC * TQ, 1], F32, kind="ExternalInput")
    amask = nc.dram_tensor("amask", [BLOC * TQ, 1], F32, kind="ExternalInput")
    wrep = nc.dram_tensor("wrep", [128, TOPK], F32, kind="ExternalInput")
    brep = nc.dram_tensor("brep", [128, 1], F32, kind="ExternalInput")
    gwrep = nc.dram_tensor("gwrep", [128, 1], F32, kind="ExternalInput")
    g4 = nc.dram_tensor("g4", [128, 4], F32, kind="ExternalInput")
    ident = nc.dram_tensor("ident", [128, 128], BF16, kind="ExternalInput")
    drow = nc.dram_tensor("drow", [1, EB], BF16, kind="ExternalInput")
    owob = nc.dram_tensor("owob_in", [1, 32], F32, kind="ExternalInput")
    y = nc.dram_tensor("y", [1, BLOC], F32, kind="ExternalOutput")

    with TileContext(nc) as tc:
        with (
            tc.tile_pool(name="persist", bufs=1) as pp,
            tc.tile_pool(name="work", bufs=3) as wp,
            tc.tile_pool(name="small", bufs=4) as sp,
            tc.tile_pool(name="psq", bufs=2, space="PSUM") as psq,
            tc.tile_pool(name="psc", bufs=4, space="PSUM") as psc,
            tc.tile_pool(name="pst", bufs=1, space="PSUM") as pst,
            tc.tile_pool(name="dram", bufs=1, space="DRAM") as dp,
        ):
            nc.gpsimd.load_library(library_config.mlp)

            # ---- persistent tiles ----
            slice_sb = pp.tile([128, RANKS, EB], BF16)
            qnT0 = pp.tile([128, B * TQ], BF16)
            qnT1 = pp.tile([128, B * TQ], BF16)
            qnT2 = pp.tile([45, B * TQ], BF16)
            gidx_sb = pp.tile([128, 16, nidx // 16], I16)
            wrep_sb = pp.tile([128, TOPK], F32)
            brep_sb = pp.tile([128, 1], F32)
            gwrep_sb = pp.tile([128, 1], F32)
            g4_sb = pp.tile([128, 4], F32)
            ident_sb = pp.tile([128, 128], BF16)
            owob_sb = pp.tile([1, 32], F32)

            cc_in = dp.tile([32, 128, 24], BF16)
            cc_out = dp.tile([32, 128, 24], BF16)

            eps_sb = pp.tile([128, 1], F32)
            nc.vector.memset(eps_sb[:], 1e-12)
            nc.sync.dma_start(out=gidx_sb[:], in_=gidx[:].rearrange("p (g w) -> p g w", g=16))
            nc.sync.dma_start(out=wrep_sb[:], in_=wrep[:])
            nc.sync.dma_start(out=brep_sb[:], in_=brep[:])
            nc.sync.dma_start(out=gwrep_sb[:], in_=gwrep[:])
            nc.sync.dma_start(out=g4_sb[:], in_=g4[:])
            nc.sync.dma_start(out=ident_sb[:], in_=ident[:])
            nc.sync.dma_start(out=owob_sb[:], in_=owob[:])

            # ---- phase A: normalize table slice into SBUF (bf16, 384-wide rows) ----
            nc.vector.memset(slice_sb[:, :, EMB:EB], 0)
            for t in range(NRANK):
                st = wp.tile([128, EMB], BF16, tag="st")
                nc.sync.dma_start(out=st[:], in_=tslice[t * 128:(t + 1) * 128, :])
                ssq = sp.tile([128, 1], F32, tag="ssq")
                sqs = wp.tile([128, EMB], F32, tag="sqs")
                nc.scalar.activation(
                    out=sqs[:], in_=st[:],
                    func=mybir.ActivationFunctionType.Square, accum_out=ssq[:],
                )
                rt = sp.tile([128, 1], F32, tag="rt")
                nc.scalar.activation(
                    out=rt[:], in_=ssq[:],
                    func=mybir.ActivationFunctionType.Sqrt, bias=eps_sb[:],
                )
                inv = sp.tile([128, 1], F32, tag="inv")
                nc.vector.reciprocal(inv[:], rt[:])
                nc.vector.tensor_scalar(
                    out=slice_sb[:, t, 0:EMB], in0=st[:],
                    scalar1=inv[:], scalar2=None, op0=MUL,
                )
            # dummy row (rank 49, partition 0): zeros with mask coordinate NEG_PAD
            nc.vector.memset(slice_sb[:, NRANK, :], 0)
            nc.sync.dma_start(out=slice_sb[0:1, NRANK, :], in_=drow[:])

            # ---- phase B: queries -> normalized, transposed qnT (301 x 4096) ----
            for qt in range(B * TQ // 128):
                qe = wp.tile([128, EMB], BF16, tag="qe")
                nc.sync.dma_start(out=qe[:], in_=qrows[qt * 128:(qt + 1) * 128, :])
                qssq = sp.tile([128, 1], F32, tag="qssq")
                qsqs = wp.tile([128, EMB], F32, tag="qsqs")
                nc.scalar.activation(
                    out=qsqs[:], in_=qe[:],
                    func=mybir.ActivationFunctionType.Square, accum_out=qssq[:],
                )
                qrt = sp.tile([128, 1], F32, tag="qrt")
                nc.scalar.activation(
                    out=qrt[:], in_=qssq[:],
                    func=mybir.ActivationFunctionType.Sqrt, bias=eps_sb[:],
                )
                qinv = sp.tile([128, 1], F32, tag="qinv")
                nc.vector.reciprocal(qinv[:], qrt[:])
                qn16 = wp.tile([128, EMB + 1], BF16, tag="qn16")
                nc.vector.tensor_scalar(
                    out=qn16[:, 0:EMB], in0=qe[:], scalar1=qinv[:], scalar2=None, op0=MUL,
                )
                nc.vector.memset(qn16[:, EMB:EMB + 1], 1.0)
                for k, (dst, w) in enumerate([(qnT0, 128), (qnT1, 128), (qnT2, 45)]):
                    tp = psq.tile([128, 128], BF16, tag="tp")
                    nc.tensor.transpose(
                        out=tp[0:w, :], in_=qn16[:, k * 128:k * 128 + w],
                        identity=ident_sb[:],
                    )
                    nc.scalar.copy(out=dst[0:w, qt * 128:(qt + 1) * 128], in_=tp[0:w, :])

            # ---- phase C: gather + cosine + per-core top-24 ----
            for g in range(16):
                gout = wp.tile([128, 8, 3, cap], BF16, tag="gout", bufs=2)
                for j in range(8):
                    nc.gpsimd.dma_gather(
                        gout[:, j, :, :], slice_sb[:],
                        gidx_sb[:, g, j * cap // 16:(j + 1) * cap // 16], cap, cap, EB,
                        transpose=True,
                        sbuf_tokens_per_rank=128,
                        sbuf_free_dim_per_rank=ROWB,
                    )
                for h in range(2):          # quads of batches
                    qd = 2 * g + h
                    cos = wp.tile([128, cap], BF16, tag="cos", bufs=3)
                    for n in range(2):
                        ps = psc.tile([128, nch], F32, tag="ps")
                        for k, kk in enumerate([128, 128, 45]):
                            for jb in range(4):
                                b = 8 * g + 4 * h + jb
                                lhs = (qnT0, qnT1, qnT2)[k]
                                nc.tensor.matmul(
                                    out=ps[32 * jb:32 * jb + 32, :],
                                    lhsT=lhs[0:kk, b * TQ:(b + 1) * TQ],
                                    rhs=gout[0:kk, 4 * h + jb, k, n * nch:(n + 1) * nch],
                                    start=(k == 0), stop=(k == 2),
                                    skip_group_check=True,
                                    tile_position=(0, 32 * jb),
                                )
                        nc.scalar.copy(out=cos[:, n * nch:(n + 1) * nch], in_=ps[:])
                    t24 = sp.tile([128, 24], BF16, tag="t24")
                    for r in range(3):
                        nc.vector.max(out=t24[:, 8 * r:8 * r + 8], in_=cos[:])
                        if r < 2:
                            nc.vector.match_replace(
                                out=cos[:], in_to_replace=t24[:, 8 * r:8 * r + 8],
                                in_values=cos[:], imm_value=NEG_REP,
                            )
                    nc.sync.dma_start(out=cc_in[qd, :, :], in_=t24[:])

            # ---- phase D: exchange candidates (AllToAll, 4 quads per peer) ----
            nc.gpsimd.collective_compute(
                "AllToAll",
                BYP,
                replica_groups=[list(range(NC))],
                ins=[cc_in[:]],
                outs=[cc_out[:]],
            )

            # ---- phase E: merge + FFW/gate tail for this core's 16 batches ----
            ps_tail = pst.tile([1, 32], F32)
            cc_view = cc_out[:].rearrange("(r j) p e -> r j p e", r=NC)
            for m in range(4):
                mt = wp.tile([128, NC, 24], BF16, tag="mt")
                nc.sync.dma_start(
                    out=mt[:], in_=cc_view[:, m, :, :].rearrange("r p e -> p r e"),
                )
                t24m = sp.tile([128, 24], BF16, tag="t24m")
                mtf = mt[:].rearrange("p r e -> p (r e)")
                for r in range(3):
                    nc.vector.max(out=t24m[:, 8 * r:8 * r + 8], in_=mtf)
                    if r < 2:
                        nc.vector.match_replace(
                            out=mtf, in_to_replace=t24m[:, 8 * r:8 * r + 8],
                            in_values=mtf, imm_value=NEG_REP,
                        )
                tk = sp.tile([128, TOPK], F32, tag="tk")
                nc.vector.tensor_copy(out=tk[:], in_=t24m[:, 0:TOPK])
                prod = sp.tile([128, TOPK], F32, tag="prod")
                nc.vector.tensor_tensor(out=prod[:], in0=tk[:], in1=wrep_sb[:], op=MUL)
                s = sp.tile([128, 1], F32, tag="s")
                nc.vector.reduce_sum(out=s[:], in_=prod[:], axis=AX)
                ffw = sp.tile([128, 1], F32, tag="ffw")
                nc.scalar.activation(
                    out=ffw[:], in_=s[:],
                    func=mybir.ActivationFunctionType.Tanh, bias=brep_sb[:, 0:1],
                )
                idf_t = sp.tile([128, 1], F32, tag="idf_t")
                nc.sync.dma_start(out=idf_t[:], in_=idf[m * 128:(m + 1) * 128, :])
                am_t = sp.tile([128, 1], F32, tag="am_t")
                nc.sync.dma_start(out=am_t[:], in_=amask[m * 128:(m + 1) * 128, :])
                gl = sp.tile([128, 1], F32, tag="gl")
                nc.vector.tensor_tensor(out=gl[:], in0=idf_t[:], in1=gwrep_sb[:], op=MUL)
                nc.vector.tensor_tensor(out=gl[:], in0=gl[:], in1=am_t[:], op=ADD)
                nc.vector.tensor_scalar_max(gl[:], gl[:], -87.0)
                ex = sp.tile([128, 1], F32, tag="ex")
                nc.scalar.activation(out=ex[:], in_=gl[:], func=mybir.ActivationFunctionType.Exp)
                pe2 = sp.tile([128, 2], F32, tag="pe2")
                nc.vector.tensor_tensor(out=pe2[:, 0:1], in0=ex[:], in1=ffw[:], op=MUL)
                nc.vector.tensor_copy(out=pe2[:, 1:2], in_=ex[:])
                nc.tensor.matmul(
                    out=ps_tail[0:1, 4 * m:4 * m + 4], lhsT=pe2[:, 0:1], rhs=g4_sb[:],
                    start=True, stop=True,
                )
                nc.tensor.matmul(
                    out=ps_tail[0:1, 16 + 4 * m:16 + 4 * m + 4], lhsT=pe2[:, 1:2], rhs=g4_sb[:],
                    start=True, stop=True,
                )
            sc = sp.tile([1, 32], F32, tag="sc")
            nc.scalar.copy(out=sc[:], in_=ps_tail[:])
            den = sp.tile([1, 16], F32, tag="den")
            nc.vector.reciprocal(den[:], sc[0:1, 16:32])
            dd = sp.tile([1, 16], F32, tag="dd")
            nc.vector.tensor_tensor(out=dd[:], in0=sc[0:1, 0:16], in1=den[:], op=MUL)
            nc.vector.tensor_tensor(out=dd[:], in0=dd[:], in1=owob_sb[0:1, 0:16], op=MUL)
            nc.vector.tensor_tensor(out=dd[:], in0=dd[:], in1=owob_sb[0:1, 16:32], op=ADD)
            out_sb = sp.tile([1, BLOC], F32, tag="out_sb")
            nc.vector.tensor_copy(out=out_sb[:], in_=dd[:])
            nc.sync.dma_start(out=y[:], in_=out_sb[:])

    if sim:
        nc.insert_bir_kernel_barrier_sem_inc()
    else:
        nc.compile()
    return nc


def _host_prep(doc, query, query_idf, emb_table, ffw_W, ffw_b, gates_W, out_W, out_b):
    doc = np.asarray(doc)
    query = np.asarray(query)
    query_idf = np.asarray(query_idf, dtype=np.float32)
    emb = np.asarray(emb_table, dtype=np.float32)
    ffw_W = np.asarray(ffw_W, dtype=np.float32)
    ffw_b = np.asarray(ffw_b, dtype=np.float32)
    gates_W = np.asarray(gates_W, dtype=np.float32)
    out_W = np.asarray(out_W, dtype=np.float32)
    out_b = np.asarray(out_b, dtype=np.float32)

    core_of = doc // VS                              # (B, TD)
    rel = (doc - core_of * VS).astype(np.int64)

    counts = np.zeros((NC, B), np.int64)
    for c in range(NC):
        counts[c] = (core_of == c).sum(axis=1)
    maxc = int(counts.max())
    cap = max(448, ((maxc + 32 + 31) // 32) * 32)
    assert cap <= 1024, f"token bucket overflow: {maxc}"

    # per-core index streams, padded with DUMMY
    gidx_all = []
    for c in range(NC):
        flat = np.full((B, cap), DUMMY, np.int64)
        for b in range(B):
            sel = rel[b][core_of[b] == c]
            flat[b, :sel.size] = sel
        nidx = 8 * cap
        per_g = flat.reshape(16, nidx)
        wrapped = np.stack(
            [per_g[g].reshape(nidx // 16, 16).T for g in range(16)], axis=1
        )  # (16, 16, nidx//16)
        tiled = np.tile(wrapped, (8, 1, 1))          # (128, 16, nidx//16)
        gidx_all.append(
            np.ascontiguousarray(
                tiled.reshape(128, 16 * (nidx // 16)).astype(np.uint16)
            ).view(np.int16)
        )

    qrows = np.ascontiguousarray(
        emb[query.reshape(-1).astype(np.int64)].astype(ml_dtypes.bfloat16)
    )

    wrep = np.ascontiguousarray(np.tile(ffw_W.reshape(1, TOPK), (128, 1)))
    brep = np.full((128, 1), float(ffw_b.reshape(-1)[0]), np.float32)
    gwrep = np.full((128, 1), float(gates_W.reshape(-1)[0]), np.float32)
    g4 = (np.arange(128)[:, None] // 32 == np.arange(4)[None, :]).astype(np.float32)
    ident = np.eye(128, dtype=ml_dtypes.bfloat16)
    drow = np.zeros((1, EB), np.float32)
    drow[0, EMB] = NEG_PAD
    drow = drow.astype(ml_dtypes.bfloat16)
    owob = np.zeros((1, 32), np.float32)
    owob[0, :16] = float(out_W.reshape(-1)[0])
    owob[0, 16:] = float(out_b.reshape(-1)[0])

    in_maps = []
    for c in range(NC):
        tslice = np.zeros((VPAD, EMB), ml_dtypes.bfloat16)
        tslice[:VS] = emb[c * VS:(c + 1) * VS].astype(ml_dtypes.bfloat16)
        qsl = query[c * BLOC:(c + 1) * BLOC].reshape(-1)
        amask = np.where(qsl == 0, -1e7, 0.0).astype(np.float32).reshape(-1, 1)
        idf_sl = np.ascontiguousarray(
            query_idf[c * BLOC:(c + 1) * BLOC].reshape(-1, 1).astype(np.float32)
        )
        in_maps.append({
            "tslice": tslice,
            "qrows": qrows,
            "gidx": gidx_all[c],
            "idf": idf_sl,
            "amask": amask,
            "wrep": wrep,
            "brep": brep,
            "gwrep": gwrep,
            "g4": g4,
            "ident": ident,
            "drow": drow,
            "owob_in": owob,
        })
    return cap, in_maps


def kernel(doc, query, query_idf, emb_table, ffw_W, ffw_b, gates_W, out_W, out_b,
           topk=20, **_unused):
    global LAST_RESULTS
    assert int(topk) == TOPK
    cap, in_maps = _host_prep(
        doc, query, query_idf, emb_table, ffw_W, ffw_b, gates_W, out_W, out_b
    )
    if cap not in _cache:
        _cache[cap] = _build(cap)
    nc = _cache[cap]
    res = run_bass_kernel_spmd(nc, in_maps, core_ids=list(range(NC)))
    LAST_RESULTS = res
    out = np.concatenate(
        [res.results[c]["y"].reshape(BLOC, 1) for c in range(NC)], axis=0
    )
    return out.astype(np.float32)


# revision 16
# speedup vs baseline: 1.0246x; 1.0246x over previous
"""DRMMTKS scoring kernel for 8 Trainium2 NeuronCores.

Strategy (vocab-sharded):
  - Each core owns a 6250-row slice of the embedding table (shipped bf16),
    normalizes it on device, and writes it to a DRAM scratch table
    (rows padded to 384 elems; a 301st coordinate carries the pad marker).
  - Doc tokens are bucketed by vocab slice on the host; each core gathers its
    tokens' normalized embeddings straight from the DRAM table with the SWDGE
    gather-transpose instruction (4 batches = 4*cap tokens per call), landing
    them in [emb, token] (matmul rhs) layout.
  - Queries are host-gathered (pure data movement), normalized + transposed on
    device; cosine similarities are computed with col-tiled matmuls
    (4 batches concurrently in the 128x128 PE array).
  - Pad tokens point at a dummy table row whose marker coordinate is -1e30
    (queries carry 1.0 there), so padded slots come out of the matmul as
    -1e30 and never reach the top-k.
  - Cosine rows are copied out of PSUM as bf16; per-core top-24 candidates
    (3 rounds of DVE max8 + match_replace) are exchanged with an AllToAll
    (each core sends exactly the 4 quads each peer's tail needs), then each
    core merges 8x24 candidates and computes the tanh-FFW + IDF-softmax tail
    for its own 16 batches.
"""

import sys

sys.path.insert(0, "/opt/trn_rl_repo")

import numpy as np
import ml_dtypes

import concourse.bass as bass
from concourse import bacc, library_config
import concourse.mybir as mybir
from concourse.tile import TileContext
from concourse.bass_utils import run_bass_kernel_spmd

# Problem shapes (hardcoded per contest contract)
V, EMB = 50000, 300
B, TQ, TD, TOPK = 128, 32, 4096, 20
NC = 8
VS = V // NC                 # 6250 vocab rows per core
NRANK = 49                   # ranks holding real vocab rows (6272 capacity)
RANKS = 50                   # +1 rank for the dummy pad row
VPAD = NRANK * 128           # 6272
DUMMY = VPAD                 # dummy row index (zero vector, mask coord -1e30)
EB = 384                     # stored row length in bf16 elems (768 bytes)
ROWB = EB * 2                # row bytes
BLOC = B // NC               # 16 batches per core in the tail
F32 = mybir.dt.float32
BF16 = mybir.dt.bfloat16
I32 = mybir.dt.int32
I16 = mybir.dt.int16
AX = mybir.AxisListType.X
MUL = mybir.AluOpType.mult
ADD = mybir.AluOpType.add
BYP = mybir.AluOpType.bypass

NEG_PAD = -1.0e30            # cos value of padded slots
NEG_REP = -2.0e30            # match_replace fill

_cache = {}

LAST_RESULTS = None          # BassKernelResults of the most recent run


def _build(cap, sim=False):
    """Trace + compile the per-core program. cap = token slots per batch."""
    nidx = 8 * cap           # gathered tokens per group (8 batches)
    assert nidx % 128 == 0 and cap % 32 == 0
    nch = cap // 2           # cos free-dim chunk (<=512)
    assert nch <= 512

    nc = bacc.Bacc("TRN2", target_bir_lowering=False, debug=False, num_devices=NC)

    tslice = nc.dram_tensor("tslice", [VPAD, EMB], BF16, kind="ExternalInput")
    qrows = nc.dram_tensor("qrows", [B * TQ, EMB], BF16, kind="ExternalInput")
    gidx = nc.dram_tensor("gidx", [128, 16 * nidx // 16], I16, kind="ExternalInput")
    idf = nc.dram_tensor("idf", [BLOC * TQ, 1], F32, kind="ExternalInput")
    amask = nc.dram_tensor("amask", [BLOC * TQ, 1], F32, kind="ExternalInput")
    wrep = nc.dram_tensor("wrep", [128, TOPK], F32, kind="ExternalInput")
    brep = nc.dram_tensor("brep", [128, 1], F32, kind="ExternalInput")
    gwrep = nc.dram_tensor("gwrep", [128, 1], F32, kind="ExternalInput")
    g4 = nc.dram_tensor("g4", [128, 4], F32, kind="ExternalInput")
    ident = nc.dram_tensor("ident", [128, 128], BF16, kind="ExternalInput")
    drow = nc.dram_tensor("drow", [1, EB], BF16, kind="ExternalInput")
    owob = nc.dram_tensor("owob_in", [1, 32], F32, kind="ExternalInput")
    y = nc.dram_tensor("y", [1, BLOC], F32, kind="ExternalOutput")

    with TileContext(nc) as tc:
        with (
            tc.tile_pool(name="persist", bufs=1) as pp,
            tc.tile_pool(name="work", bufs=3) as wp,
            tc.tile_pool(name="small", bufs=4) as sp,
            tc.tile_pool(name="psq", bufs=2, space="PSUM") as psq,
            tc.tile_pool(name="psc", bufs=4, space="PSUM") as psc,
            tc.tile_pool(name="pst", bufs=1, space="PSUM") as pst,
            tc.tile_pool(name="dram", bufs=1, space="DRAM") as dp,
        ):
            nc.gpsimd.load_library(library_config.mlp)

            # ---- persistent tiles ----
            slice_sb = pp.tile([128, RANKS, EB], BF16)
            qnT0 = pp.tile([128, B * TQ], BF16)
            qnT1 = pp.tile([128, B * TQ], BF16)
            qnT2 = pp.tile([45, B * TQ], BF16)
            gidx_sb = pp.tile([128, 16, nidx // 16], I16)
            wrep_sb = pp.tile([128, TOPK], F32)
            brep_sb = pp.tile([128, 1], F32)
            gwrep_sb = pp.tile([128, 1], F32)
            g4_sb = pp.tile([128, 4], F32)
            ident_sb = pp.tile([128, 128], BF16)
            owob_sb = pp.tile([1, 32], F32)

            cc_in = dp.tile([32, 128, 24], BF16)
            cc_out = dp.tile([32, 128, 24], BF16)

            eps_sb = pp.tile([128, 1], F32)
            nc.vector.memset(eps_sb[:], 1e-12)
            nc.sync.dma_start(out=gidx_sb[:], in_=gidx[:].rearrange("p (g w) -> p g w", g=16))
            nc.sync.dma_start(out=wrep_sb[:], in_=wrep[:])
            nc.sync.dma_start(out=brep_sb[:], in_=brep[:])
            nc.sync.dma_start(out=gwrep_sb[:], in_=gwrep[:])
            nc.sync.dma_start(out=g4_sb[:], in_=g4[:])
            nc.sync.dma_start(out=ident_sb[:], in_=ident[:])
            nc.sync.dma_start(out=owob_sb[:], in_=owob[:])

            # ---- phase A: normalize table slice into SBUF (bf16, 384-wide rows) ----
            nc.vector.memset(slice_sb[:, :, EMB:EB], 0)
            CH = 7                       # ranks per chunk (49 = 7*7)
            for t0 in range(0, NRANK, CH):
                st = wp.tile([128, CH, EMB], BF16, tag="st")
                nc.sync.dma_start(
                    out=st[:],
                    in_=tslice[t0 * 128:(t0 + CH) * 128, :]
                        .rearrange("(t p) e -> p t e", p=128),
                )
                sqs = wp.tile([128, CH, EMB], F32, tag="sqs")
                nc.scalar.activation(
                    out=sqs[:], in_=st[:],
                    func=mybir.ActivationFunctionType.Square,
                )
                ssq = sp.tile([128, CH, 1], F32, tag="ssq")
                nc.vector.reduce_sum(out=ssq[:], in_=sqs[:], axis=AX)
                rt = sp.tile([128, CH], F32, tag="rt")
                nc.scalar.activation(
                    out=rt[:], in_=ssq[:].rearrange("p t o -> p (t o)"),
                    func=mybir.ActivationFunctionType.Sqrt, bias=eps_sb[:],
                )
                inv = sp.tile([128, CH, 1], F32, tag="inv")
                nc.vector.reciprocal(inv[:].rearrange("p t o -> p (t o)"), rt[:])
                nc.vector.tensor_tensor(
                    out=slice_sb[:, t0:t0 + CH, 0:EMB], in0=st[:],
                    in1=inv[:].to_broadcast([128, CH, EMB]), op=MUL,
                )
            # dummy row (rank 49, partition 0): zeros with mask coordinate NEG_PAD
            nc.vector.memset(slice_sb[:, NRANK, :], 0)
            nc.sync.dma_start(out=slice_sb[0:1, NRANK, :], in_=drow[:])

            # ---- phase B: queries -> normalized, transposed qnT (301 x 4096) ----
            for qt in range(B * TQ // 128):
                qe = wp.tile([128, EMB], BF16, tag="qe")
                nc.sync.dma_start(out=qe[:], in_=qrows[qt * 128:(qt + 1) * 128, :])
                qssq = sp.tile([128, 1], F32, tag="qssq")
                qsqs = wp.tile([128, EMB], F32, tag="qsqs")
                nc.scalar.activation(
                    out=qsqs[:], in_=qe[:],
                    func=mybir.ActivationFunctionType.Square, accum_out=qssq[:],
                )
                qrt = sp.tile([128, 1], F32, tag="qrt")
                nc.scalar.activation(
                    out=qrt[:], in_=qssq[:],
                    func=mybir.ActivationFunctionType.Sqrt, bias=eps_sb[:],
                )
                qinv = sp.tile([128, 1], F32, tag="qinv")
                nc.vector.reciprocal(qinv[:], qrt[:])
                qn16 = wp.tile([128, EMB + 1], BF16, tag="qn16")
                nc.vector.tensor_scalar(
                    out=qn16[:, 0:EMB], in0=qe[:], scalar1=qinv[:], scalar2=None, op0=MUL,
                )
                nc.vector.memset(qn16[:, EMB:EMB + 1], 1.0)
                for k, (dst, w) in enumerate([(qnT0, 128), (qnT1, 128), (qnT2, 45)]):
                    tp = psq.tile([128, 128], BF16, tag="tp")
                    nc.tensor.transpose(
                        out=tp[0:w, :], in_=qn16[:, k * 128:k * 128 + w],
                        identity=ident_sb[:],
                    )
                    nc.scalar.copy(out=dst[0:w, qt * 128:(qt + 1) * 128], in_=tp[0:w, :])

            # ---- phase C: gather + cosine + per-core top-24 ----
            for g in range(16):
                gout = wp.tile([128, 8, 3, cap], BF16, tag="gout", bufs=2)
                for j in range(8):
                    nc.gpsimd.dma_gather(
                        gout[:, j, :, :], slice_sb[:],
                        gidx_sb[:, g, j * cap // 16:(j + 1) * cap // 16], cap, cap, EB,
                        transpose=True,
                        sbuf_tokens_per_rank=128,
                        sbuf_free_dim_per_rank=ROWB,
                    )
                for h in range(2):          # quads of batches
                    qd = 2 * g + h
                    cos = wp.tile([128, cap], BF16, tag="cos", bufs=3)
                    for n in range(2):
                        ps = psc.tile([128, nch], F32, tag="ps")
                        for k, kk in enumerate([128, 128, 45]):
                            for jb in range(4):
                                b = 8 * g + 4 * h + jb
                                lhs = (qnT0, qnT1, qnT2)[k]
                                nc.tensor.matmul(
                                    out=ps[32 * jb:32 * jb + 32, :],
                                    lhsT=lhs[0:kk, b * TQ:(b + 1) * TQ],
                                    rhs=gout[0:kk, 4 * h + jb, k, n * nch:(n + 1) * nch],
                                    start=(k == 0), stop=(k == 2),
                                    skip_group_check=True,
                                    tile_position=(0, 32 * jb),
                                )
                        nc.scalar.copy(out=cos[:, n * nch:(n + 1) * nch], in_=ps[:])
                    t24 = sp.tile([128, 24], BF16, tag="t24")
                    for r in range(3):
                        nc.vector.max(out=t24[:, 8 * r:8 * r + 8], in_=cos[:])
                        if r < 2:
                            nc.vector.match_replace(
                                out=cos[:], in_to_replace=t24[:, 8 * r:8 * r + 8],
                                in_values=cos[:], imm_value=NEG_REP,
                            )
                    nc.sync.dma_start(out=cc_in[qd, :, :], in_=t24[:])

            # ---- phase D: exchange candidates (AllToAll, 4 quads per peer) ----
            nc.gpsimd.collective_compute(
                "AllToAll",
                BYP,
                replica_groups=[list(range(NC))],
                ins=[cc_in[:]],
                outs=[cc_out[:]],
            )

            # ---- phase E: merge + FFW/gate tail for this core's 16 batches ----
            ps_tail = pst.tile([1, 32], F32)
            cc_view = cc_out[:].rearrange("(r j) p e -> r j p e", r=NC)
            for m in range(4):
                mt = wp.tile([128, NC, 24], BF16, tag="mt")
                nc.sync.dma_start(
                    out=mt[:], in_=cc_view[:, m, :, :].rearrange("r p e -> p r e"),
                )
                t24m = sp.tile([128, 24], BF16, tag="t24m")
                mtf = mt[:].rearrange("p r e -> p (r e)")
                for r in range(3):
                    nc.vector.max(out=t24m[:, 8 * r:8 * r + 8], in_=mtf)
                    if r < 2:
                        nc.vector.match_replace(
                            out=mtf, in_to_replace=t24m[:, 8 * r:8 * r + 8],
                            in_values=mtf, imm_value=NEG_REP,
                        )
                tk = sp.tile([128, TOPK], F32, tag="tk")
                nc.vector.tensor_copy(out=tk[:], in_=t24m[:, 0:TOPK])
                prod = sp.tile([128, TOPK], F32, tag="prod")
                nc.vector.tensor_tensor(out=prod[:], in0=tk[:], in1=wrep_sb[:], op=MUL)
                s = sp.tile([128, 1], F32, tag="s")
                nc.vector.reduce_sum(out=s[:], in_=prod[:], axis=AX)
                ffw = sp.tile([128, 1], F32, tag="ffw")
                nc.scalar.activation(
                    out=ffw[:], in_=s[:],
                    func=mybir.ActivationFunctionType.Tanh, bias=brep_sb[:, 0:1],
                )
                idf_t = sp.tile([128, 1], F32, tag="idf_t")
                nc.sync.dma_start(out=idf_t[:], in_=idf[m * 128:(m + 1) * 128, :])
                am_t = sp.tile([128, 1], F32, tag="am_t")
                nc.sync.dma_start(out=am_t[:], in_=amask[m * 128:(m + 1) * 128, :])
                gl = sp.tile([128, 1], F32, tag="gl")
                nc.vector.tensor_tensor(out=gl[:], in0=idf_t[:], in1=gwrep_sb[:], op=MUL)
                nc.vector.tensor_tensor(out=gl[:], in0=gl[:], in1=am_t[:], op=ADD)
                nc.vector.tensor_scalar_max(gl[:], gl[:], -87.0)
                ex = sp.tile([128, 1], F32, tag="ex")
                nc.scalar.activation(out=ex[:], in_=gl[:], func=mybir.ActivationFunctionType.Exp)
                pe2 = sp.tile([128, 2], F32, tag="pe2")
                nc.vector.tensor_tensor(out=pe2[:, 0:1], in0=ex[:], in1=ffw[:], op=MUL)
                nc.vector.tensor_copy(out=pe2[:, 1:2], in_=ex[:])
                nc.tensor.matmul(
                    out=ps_tail[0:1, 4 * m:4 * m + 4], lhsT=pe2[:, 0:1], rhs=g4_sb[:],
                    start=True, stop=True,
                )
                nc.tensor.matmul(
                    out=ps_tail[0:1, 16 + 4 * m:16 + 4 * m + 4], lhsT=pe2[:, 1:2], rhs=g4_sb[:],
                    start=True, stop=True,
                )
            sc = sp.tile([1, 32], F32, tag="sc")
            nc.scalar.copy(out=sc[:], in_=ps_tail[:])
            den = sp.tile([1, 16], F32, tag="den")
            nc.vector.reciprocal(den[:], sc[0:1, 16:32])
            dd = sp.tile([1, 16], F32, tag="dd")
            nc.vector.tensor_tensor(out=dd[:], in0=sc[0:1, 0:16], in1=den[:], op=MUL)
            nc.vector.tensor_tensor(out=dd[:], in0=dd[:], in1=owob_sb[0:1, 0:16], op=MUL)
            nc.vector.tensor_tensor(out=dd[:], in0=dd[:], in1=owob_sb[0:1, 16:32], op=ADD)
            out_sb = sp.tile([1, BLOC], F32, tag="out_sb")
            nc.vector.tensor_copy(out=out_sb[:], in_=dd[:])
            nc.sync.dma_start(out=y[:], in_=out_sb[:])

    if sim:
        nc.insert_bir_kernel_barrier_sem_inc()
    else:
        nc.compile()
    return nc


def _host_prep(doc, query, query_idf, emb_table, ffw_W, ffw_b, gates_W, out_W, out_b):
    doc = np.asarray(doc)
    query = np.asarray(query)
    query_idf = np.asarray(query_idf, dtype=np.float32)
    emb = np.asarray(emb_table, dtype=np.float32)
    ffw_W = np.asarray(ffw_W, dtype=np.float32)
    ffw_b = np.asarray(ffw_b, dtype=np.float32)
    gates_W = np.asarray(gates_W, dtype=np.float32)
    out_W = np.asarray(out_W, dtype=np.float32)
    out_b = np.asarray(out_b, dtype=np.float32)

    core_of = doc // VS                              # (B, TD)
    rel = (doc - core_of * VS).astype(np.int64)

    counts = np.zeros((NC, B), np.int64)
    for c in range(NC):
        counts[c] = (core_of == c).sum(axis=1)
    maxc = int(counts.max())
    cap = max(448, ((maxc + 32 + 31) // 32) * 32)
    assert cap <= 1024, f"token bucket overflow: {maxc}"

    # per-core index streams, padded with DUMMY
    gidx_all = []
    for c in range(NC):
        flat = np.full((B, cap), DUMMY, np.int64)
        for b in range(B):
            sel = rel[b][core_of[b] == c]
            flat[b, :sel.size] = sel
        nidx = 8 * cap
        per_g = flat.reshape(16, nidx)
        wrapped = np.stack(
            [per_g[g].reshape(nidx // 16, 16).T for g in range(16)], axis=1
        )  # (16, 16, nidx//16)
        tiled = np.tile(wrapped, (8, 1, 1))          # (128, 16, nidx//16)
        gidx_all.append(
            np.ascontiguousarray(
                tiled.reshape(128, 16 * (nidx // 16)).astype(np.uint16)
            ).view(np.int16)
        )

    qrows = np.ascontiguousarray(
        emb[query.reshape(-1).astype(np.int64)].astype(ml_dtypes.bfloat16)
    )

    wrep = np.ascontiguousarray(np.tile(ffw_W.reshape(1, TOPK), (128, 1)))
    brep = np.full((128, 1), float(ffw_b.reshape(-1)[0]), np.float32)
    gwrep = np.full((128, 1), float(gates_W.reshape(-1)[0]), np.float32)
    g4 = (np.arange(128)[:, None] // 32 == np.arange(4)[None, :]).astype(np.float32)
    ident = np.eye(128, dtype=ml_dtypes.bfloat16)
    drow = np.zeros((1, EB), np.float32)
    drow[0, EMB] = NEG_PAD
    drow = drow.astype(ml_dtypes.bfloat16)
    owob = np.zeros((1, 32), np.float32)
    owob[0, :16] = float(out_W.reshape(-1)[0])
    owob[0, 16:] = float(out_b.reshape(-1)[0])

    in_maps = []
    for c in range(NC):
        tslice = np.zeros((VPAD, EMB), ml_dtypes.bfloat16)
        tslice[:VS] = emb[c * VS:(c + 1) * VS].astype(ml_dtypes.bfloat16)
        qsl = query[c * BLOC:(c + 1) * BLOC].reshape(-1)
        amask = np.where(qsl == 0, -1e7, 0.0).astype(np.float32).reshape(-1, 1)
        idf_sl = np.ascontiguousarray(
            query_idf[c * BLOC:(c + 1) * BLOC].reshape(-1, 1).astype(np.float32)
        )
        in_maps.append({
            "tslice": tslice,
            "qrows": qrows,
            "gidx": gidx_all[c],
            "idf": idf_sl,
            "amask": amask,
            "wrep": wrep,
            "brep": brep,
            "gwrep": gwrep,
            "g4": g4,
            "ident": ident,
            "drow": drow,
            "owob_in": owob,
        })
    return cap, in_maps


def kernel(doc, query, query_idf, emb_table, ffw_W, ffw_b, gates_W, out_W, out_b,
           topk=20, **_unused):
    global LAST_RESULTS
    assert int(topk) == TOPK
    cap, in_maps = _host_prep(
        doc, query, query_idf, emb_table, ffw_W, ffw_b, gates_W, out_W, out_b
    )
    if cap not in _cache:
        _cache[cap] = _build(cap)
    nc = _cache[cap]
    res = run_bass_kernel_spmd(nc, in_maps, core_ids=list(range(NC)))
    LAST_RESULTS = res
    out = np.concatenate(
        [res.results[c]["y"].reshape(BLOC, 1) for c in range(NC)], axis=0
    )
    return out.astype(np.float32)
